# revision 22
# baseline (speedup 1.0000x reference)
"""DGCNN-style EdgeConv point-cloud network, whole batch on ONE NeuronCore.

Math trick: edge = [center, neigh-center] @ W decomposes as
    h[n,k] = center[n] @ (Wt - Wb) + neigh[n,k] @ Wb        (Wt = W[:C], Wb = W[C:])
so per-layer work collapses to two point-level matmuls (A = F@(Wt-Wb), Bm = F@Wb)
plus a gather of Bm rows by kNN index and a max over the 16 neighbors:
    h_max[n] = A[n] + max_k Bm[idx[n,k]].
Biases fold into the (training-mode) BN shift.

Why one core: the per-call dispatch cost of this environment scales with
(#input bindings x #cores); device compute is tiny by comparison.  So all
4 clouds run on core 0 (BN statistics accumulate locally in PSUM across the
clouds - bit-exact batch stats, no collectives), the 17 input tensors are
packed host-side into a single flat f32 blob (one binding), and the output
is one [4, 512] tensor.

Embedding layer: BN scale is positive and leaky-relu is monotone increasing,
so per-channel BN+lrelu commute with the global max pool.  We therefore
compute e_T = We^T @ ft3 in transposed chunks, reduce sum / sum-sq / max per
channel on the fly, and apply BN+lrelu to the [512]-vector of maxes only -
the full (4096, 512) embedding is never materialized.
"""

import numpy as np

import concourse.bass as bass
import concourse.masks as masks
import concourse.tile as tile
from concourse import bacc, mybir
from concourse.bass_utils import run_bass_kernel_spmd

F32 = mybir.dt.float32
BF16 = mybir.dt.bfloat16
U16 = mybir.dt.uint16
I16 = mybir.dt.int16

B, N, D, KNN = 4, 1024, 3, 16
FEATURE_DIMS = [64, 128, 256]
EMB = 512
NEG = -1.0e30
EPS = 1e-5
SLOPE = 0.2
NPTS = B * N               # BN denominator: 4 clouds x 1024 pts
NT = N // 128              # 8 row tiles of 128 points
GCHUNKS = 8                # gather chunks per layer per cloud
IDX_PER_CHUNK = N * KNN // GCHUNKS
REPEAT = 1
USE_CC = True              # unused (kept for test.py env-flag compat)
USE_GATHER = True
LRELU_ACT = False

# ---- blob layout (order matches _in_maps) ----
_SIZES = [
    ("xyz", B * N * D),
    ("W1", 2 * 3 * 64), ("b1", 64), ("g1", 64), ("be1", 64),
    ("W2", 2 * 64 * 128), ("b2", 128), ("g2", 128), ("be2", 128),
    ("W3", 2 * 128 * 256), ("b3", 256), ("g3", 256), ("be3", 256),
    ("We", 256 * 512), ("bse", 512), ("ge", 512), ("bee", 512),
]
OFF = {}
_o = 0
for _nm, _sz in _SIZES:
    OFF[_nm] = _o
    _o += _sz
BLOB = _o


def _canon_out(ap2d, q):
    """Strided out-view placing natural tile-q columns (m = 0..127, point
    n = 128q + m) at canonical positions c(n) = ((n%128)//16)*128 + 16*(n//128)
    + n%16 = (m//16)*128 + 16*q + (m%16): dims [(128,8) m//16, (1,16) m%16]."""
    return bass.AP(tensor=ap2d.tensor, offset=ap2d.offset + 16 * q,
                   ap=[list(ap2d.ap[0]), [128, 8], [1, 16]])


def _sigma_out(ap2d):
    """Strided out-view writing natural column m of tile r to position
    r*128 + sigma^-1(m), sigma^-1(m) = 8*(m%16) + m//16, so that psum row p
    of the distance matmul holds point n = r*128 + sigma(p),
    sigma(p) = 16*(p%8) + p//8."""
    return bass.AP(tensor=ap2d.tensor, offset=ap2d.offset,
                   ap=[list(ap2d.ap[0]), [128, 8], [1, 8], [8, 16]])


def build_program(nc, tc, tensors, ctx):
    blob = tensors["blob"].ap()
    out_ap = tensors["out"].ap()

    def blob2d(off, rows, cols):
        return bass.AP(tensor=blob.tensor, offset=blob.offset + off,
                       ap=[[cols, rows], [1, cols]])

    cpool = ctx.enter_context(tc.tile_pool(name="const", bufs=1))
    tpool = ctx.enter_context(tc.tile_pool(name="topk", bufs=2))
    apool = ctx.enter_context(tc.tile_pool(name="acts", bufs=2))
    spool = ctx.enter_context(tc.tile_pool(name="smax", bufs=2))
    bpool = ctx.enter_context(tc.tile_pool(name="bm", bufs=2))
    qpool = ctx.enter_context(tc.tile_pool(name="hp", bufs=4))
    sqpool = ctx.enter_context(tc.tile_pool(name="sq", bufs=2))
    gpool = ctx.enter_context(tc.tile_pool(name="gath", bufs=2))
    fpool = ctx.enter_context(tc.tile_pool(name="ft", bufs=12))
    mpool = ctx.enter_context(tc.tile_pool(name="misc", bufs=2))
    bmdram = ctx.enter_context(tc.tile_pool(name="bmdram", bufs=4, space="DRAM"))

    pab = ctx.enter_context(tc.tile_pool(name="pab", bufs=2, space="PSUM"))
    pT = ctx.enter_context(tc.tile_pool(name="pT", bufs=1, space="PSUM"))
    pstat = ctx.enter_context(tc.tile_pool(name="pstat", bufs=2, space="PSUM"))

    # ---------------- constants / weights ----------------
    ident = cpool.tile([128, 128], F32)
    masks.make_identity(nc, ident[:])
    ones_col = cpool.tile([128, 1], F32)
    nc.gpsimd.memset(ones_col[:], 1.0)
    ones_row = cpool.tile([1, 128], F32)
    nc.gpsimd.memset(ones_row[:], 1.0)
    neg_ones3 = cpool.tile([3, 1], F32)
    nc.gpsimd.memset(neg_ones3[:], -1.0)

    # per-layer weight tiles: Wt/Wb K-chunks from blob slices
    wsb = {}
    for li, (cin, cout) in enumerate(zip([3] + FEATURE_DIMS[:-1], FEATURE_DIMS), start=1):
        base = OFF[f"W{li}"]
        nch = (cin + 127) // 128
        wt_tiles, wb_tiles = [], []
        for kc in range(nch):
            rows = min(128, cin - kc * 128)
            wt = cpool.tile([rows, cout], F32, tag=f"Wt{li}_{kc}")
            wb = cpool.tile([rows, cout], F32, tag=f"Wb{li}_{kc}")
            nc.sync.dma_start(wt[:], blob2d(base + kc * 128 * cout, rows, cout))
            nc.sync.dma_start(wb[:], blob2d(base + (cin + kc * 128) * cout, rows, cout))
            wt_tiles.append(wt)
            wb_tiles.append(wb)
        wsb[li] = (wt_tiles, wb_tiles)
    we_tiles = []
    for j in range(2):
        tf = cpool.tile([128, EMB], F32, tag=f"We_{j}")
        nc.sync.dma_start(tf[:], blob2d(OFF["We"] + j * 128 * EMB, 128, EMB))
        we_tiles.append(tf)

    vec_sb = {}
    for name in ["b1", "g1", "be1", "b2", "g2", "be2", "b3", "g3", "be3"]:
        c = dict(_SIZES)[name]
        t = cpool.tile([1, c], F32, tag=f"vec_{name}")
        nc.sync.dma_start(t[:], blob2d(OFF[name], 1, c))
        vec_sb[name] = t
    # embedding-layer vectors in transposed per-partition layout [128, 4]
    vecT_sb = {}
    for name in ["bse", "ge", "bee"]:
        t = cpool.tile([128, 4], F32, tag=f"vecT_{name}")
        src = bass.AP(tensor=blob.tensor, offset=blob.offset + OFF[name],
                      ap=[[1, 128], [128, 4]])
        nc.sync.dma_start(t[:], src)
        vecT_sb[name] = t

    # Wd = Wt - Wb chunks ([C_in_chunk, C_out] each)
    def make_wd(li, cin, cout):
        wt_tiles, wb_tiles = wsb[li]
        chunks = []
        for kc, (wt, wb) in enumerate(zip(wt_tiles, wb_tiles)):
            rows = wt.shape[0]
            wd = cpool.tile([rows, cout], F32, tag=f"wd{li}_{kc}")
            nc.vector.tensor_sub(wd[:], wt[:], wb[:])
            chunks.append((wd[:], wb[:]))  # (Wd, Wb)
        return chunks

    wd_chunks = {1: make_wd(1, 3, 64), 2: make_wd(2, 64, 128), 3: make_wd(3, 128, 256)}

    # ---------------- per-cloud kNN: xyz -> wrapped gather indices ----------
    wrappeds = {}
    xtcs = {}

    def emit_knn(b):
        xyz_nat = cpool.tile([128, 8, 3], F32, tag=f"xyznat{b}")
        src = bass.AP(tensor=blob.tensor, offset=blob.offset + OFF["xyz"] + b * N * D,
                      ap=[[3, 128], [384, 8], [1, 3]])
        nc.sync.dma_start(xyz_nat[:], src)
        XT = cpool.tile([3, N], F32, tag="XT", bufs=1)
        XTc = cpool.tile([3, N], F32, tag=f"XTc{b}")
        for q in range(NT):
            ps = pT.tile([3, 128], F32, tag="pT")
            nc.tensor.transpose(ps[:], xyz_nat[:, q, :], ident[:])
            nc.scalar.copy(XT[:][:, q * 128:(q + 1) * 128], ps[:])
            nc.scalar.copy(_canon_out(XTc[:], q), ps[:])

        # squared norms; matmul operands for negD = 2<xn,xm> - |xm|^2
        xtsq = cpool.tile([3, N], F32, tag="xtsq", bufs=1)
        nc.scalar.square(xtsq[:], XT[:])
        rhs4 = cpool.tile([4, N], F32, tag="rhs4", bufs=1)
        nc.vector.tensor_copy(rhs4[:][0:3, :], XT[:])
        nsq = sqpool.tile([1, N], F32, tag="nsq", bufs=1)
        for half in range(2):
            psn = pstat.tile([1, 512], F32, tag="psn")
            nc.tensor.matmul(psn[:], neg_ones3[:], xtsq[:][:, half * 512:(half + 1) * 512])
            nc.scalar.copy(nsq[:][:, half * 512:(half + 1) * 512], psn[:])
        nc.sync.dma_start(rhs4[:][3:4, :], nsq[:])
        lhsT4 = cpool.tile([4, N], F32, tag="lhsT4", bufs=1)
        nc.gpsimd.memset(lhsT4[:], 1.0)
        nc.scalar.mul(_sigma_out(lhsT4[:][0:3, :]), XT[:], 2.0)

        # top-16 neighbors
        idx_all = cpool.tile([128, 128], U16, tag=f"idxall{b}")
        for r in range(NT):
            lhs_ap = lhsT4[:][:, r * 128:(r + 1) * 128]
            negD = tpool.tile([128, N], F32, tag="negD", bufs=1)
            for hh in range(2):
                psD = pab.tile([128, 512], F32, tag="psab")
                nc.tensor.matmul(psD[:], lhs_ap, rhs4[:][:, hh * 512:(hh + 1) * 512])
                nc.scalar.copy(negD[:, hh * 512:(hh + 1) * 512], psD[:])
            vals = tpool.tile([128, 16], F32, tag="vals")
            nc.vector.max(vals[:, 0:8], negD[:])
            nc.vector.max_index(idx_all[:][:, r:r + 57:8], vals[:, 0:8], negD[:])
            nc.vector.match_replace(negD[:], vals[:, 0:8], negD[:], NEG)
            nc.vector.max(vals[:, 8:16], negD[:])
            nc.vector.max_index(idx_all[:][:, 64 + r:64 + r + 57:8], vals[:, 8:16], negD[:])

        # wrapped index layout for dma_gather: [16 partitions, 1024] u16, x8
        wrapped = cpool.tile([128, N * KNN // 16], U16, tag=f"wrapped{b}")
        for k in range(8):
            nc.sync.dma_start(wrapped[:][16 * k:16 * (k + 1), :], idx_all[:])
        wrappeds[b] = wrapped
        xtcs[b] = XTc

    # ---------------- layer core (per cloud), stats accumulate over clouds --
    def emit_layer_cloud(b, ft_chunks, cin, cout, li, ps1, ps2):
        chunks = wd_chunks[li]
        bmdt = BF16 if cout * 2 % 256 == 0 else F32   # gather rows need 256B mult

        hp = qpool.tile([128, 8, cout], F32, tag="hp", bufs=4)
        Bm = bpool.tile([128, 8, cout], bmdt, tag="Bm")
        for g in range(8):
            gs = slice(g * 128, (g + 1) * 128)
            psA = pab.tile([128, cout], F32, tag="psab")
            for kc, (wd, _) in enumerate(chunks):
                nc.tensor.matmul(psA[:], ft_chunks[kc][:, gs], wd[:],
                                 start=(kc == 0), stop=(kc == len(chunks) - 1))
            nc.scalar.copy(hp[:, g, :], psA[:])
            psB = pab.tile([128, cout], F32, tag="psab")
            for kc, (_, wb) in enumerate(chunks):
                nc.tensor.matmul(psB[:], ft_chunks[kc][:, gs], wb[:],
                                 start=(kc == 0), stop=(kc == len(chunks) - 1))
            nc.scalar.copy(Bm[:, g, :], psB[:])

        # scatter canonical tiles to natural DRAM rows n = 128*(P//16)+16g+P%16
        bm_d = bmdram.tile([N, cout], bmdt, tag="bmd")
        for g in range(8):
            dst = bass.AP(tensor=bm_d.tensor, offset=bm_d.offset + 16 * g * cout,
                          ap=[[128 * cout, 8], [cout, 16], [1, cout]])
            nc.sync.dma_start(dst, Bm[:, g, :])

        # gather + max over 16 neighbors; h_pre[:, g] = A[:, g] + S[:, g]
        wrapped = wrappeds[b]
        wslice = N * KNN // 16 // GCHUNKS
        for cc in range(GCHUNKS):
            gt = gpool.tile([128, IDX_PER_CHUNK // 128, cout], bmdt, tag="gath")
            nc.gpsimd.dma_gather(
                gt[:], bm_d[:],
                wrapped[:][:, cc * wslice:(cc + 1) * wslice].bitcast(I16),
                num_idxs=IDX_PER_CHUNK, num_idxs_reg=IDX_PER_CHUNK,
                elem_size=cout, single_packet=False)
            st = spool.tile([128, cout], F32, tag="S")
            nc.vector.tensor_reduce(
                st[:],
                gt[:].rearrange("p (gl t) c -> p gl c t", t=16),
                axis=mybir.AxisListType.X, op=mybir.AluOpType.max)
            nc.vector.tensor_add(hp[:, cc, :], hp[:, cc, :], st[:])

        # stats over points: PE ones-trick, accumulated across the 4 clouds
        for g in range(8):
            nc.tensor.matmul(ps1[:], ones_col[:], hp[:, g, :],
                             start=(b == 0 and g == 0), stop=(b == B - 1 and g == 7))
        for g in range(8):
            sg = sqpool.tile([128, cout], F32, tag="sqg")
            nc.scalar.square(sg[:], hp[:, g, :])
            nc.tensor.matmul(ps2[:], ones_col[:], sg[:],
                             start=(b == 0 and g == 0), stop=(b == B - 1 and g == 7))
        return hp

    def stats_to_scaleshift(ps1, ps2, cout, li):
        """Accumulated sum / sum-sq -> transposed per-partition BN scale/shift
        tiles [128, nch]."""
        bname, gname, bename = f"b{li}", f"g{li}", f"be{li}"
        ss = mpool.tile([1, 2 * cout], F32, tag="ss", bufs=1)
        mv = mpool.tile([1, 2 * cout], F32, tag="mv", bufs=1)
        nc.scalar.copy(mv[:, 0:cout], ps1[:])
        nc.scalar.copy(mv[:, cout:2 * cout], ps2[:])
        nc.vector.tensor_scalar_mul(mv[:], mv[:], 1.0 / NPTS)       # [mean | E[x^2]]
        mean, ex2 = mv[:, 0:cout], mv[:, cout:2 * cout]
        msq = mpool.tile([1, cout], F32, tag="msq", bufs=1)
        var = mpool.tile([1, cout], F32, tag="var", bufs=1)
        nc.vector.tensor_mul(msq[:], mean, mean)
        nc.vector.scalar_tensor_tensor(var[:], ex2, EPS, msq[:],
                                       op0=mybir.AluOpType.add,
                                       op1=mybir.AluOpType.subtract)
        nc.scalar.activation(msq[:], var[:], mybir.ActivationFunctionType.Sqrt)
        nc.vector.reciprocal(var[:], msq[:])                         # 1/sqrt(var+eps)
        scale_ap, shift_ap = ss[:, 0:cout], ss[:, cout:2 * cout]
        nc.vector.tensor_mul(scale_ap, var[:], vec_sb[gname][:])
        nc.vector.tensor_add(msq[:], mean, vec_sb[bname][:])         # mean + b
        nc.vector.tensor_mul(msq[:], msq[:], scale_ap)
        nc.vector.tensor_sub(shift_ap, vec_sb[bename][:], msq[:])

        # transpose scale/shift to per-partition layout [128, nch]
        nch = (cout + 127) // 128
        ssT_s = mpool.tile([128, nch], F32, tag="ssT_s")
        ssT_b = mpool.tile([128, nch], F32, tag="ssT_b")
        for oc in range(nch):
            cw = min(128, cout - oc * 128)
            pss = pT.tile([128, 128], F32, tag="pT")
            nc.tensor.matmul(pss[:cw, 0:1], ss[:, oc * 128:oc * 128 + cw],
                             ones_row[:][:, 0:1])
            nc.scalar.copy(ssT_s[0:cw, oc:oc + 1], pss[:cw, 0:1])
            psb = pT.tile([128, 128], F32, tag="pT")
            nc.tensor.matmul(psb[:cw, 0:1], ss[:, cout + oc * 128:cout + oc * 128 + cw],
                             ones_row[:][:, 0:1])
            nc.scalar.copy(ssT_b[0:cw, oc:oc + 1], psb[:cw, 0:1])
        return ssT_s, ssT_b

    def transpose_apply(hp, ssT_s, ssT_b, cout, b, li):
        """Transpose hp and apply fused BN + leaky relu -> ft chunks."""
        nch = (cout + 127) // 128
        fts = []
        for oc in range(nch):
            cw = min(128, cout - oc * 128)
            ft = fpool.tile([cw, N], F32, tag="ft")
            for g in range(8):
                pst = pT.tile([128, 128], F32, tag="pT")
                nc.tensor.transpose(pst[:cw, :], hp[:, g, oc * 128:oc * 128 + cw],
                                    ident[:])
                nc.scalar.activation(ft[:][:, g * 128:(g + 1) * 128], pst[:cw, :],
                                     mybir.ActivationFunctionType.Identity,
                                     bias=ssT_b[0:cw, oc:oc + 1],
                                     scale=ssT_s[0:cw, oc:oc + 1])
            v = ft[:]
            nc.vector.scalar_tensor_tensor(v, v, SLOPE, v,
                                           op0=mybir.AluOpType.mult,
                                           op1=mybir.AluOpType.max)
            fts.append(ft[:])
        return fts

    # ---------------- the three EdgeConv layers ----------------
    # L1 for cloud b is emitted right after kNN of cloud b+1, overlapping the
    # DVE-bound top-k with layer-1 matmul/gather work on other engines.
    ps1 = pstat.tile([1, 64], F32, tag="pstat")
    ps2 = pstat.tile([1, 64], F32, tag="pstat")
    hps = {}
    emit_knn(0)
    emit_knn(1)
    hps[0] = emit_layer_cloud(0, [xtcs[0][:]], 3, 64, 1, ps1, ps2)
    emit_knn(2)
    hps[1] = emit_layer_cloud(1, [xtcs[1][:]], 3, 64, 1, ps1, ps2)
    emit_knn(3)
    hps[2] = emit_layer_cloud(2, [xtcs[2][:]], 3, 64, 1, ps1, ps2)
    hps[3] = emit_layer_cloud(3, [xtcs[3][:]], 3, 64, 1, ps1, ps2)
    ssT_s, ssT_b = stats_to_scaleshift(ps1, ps2, 64, 1)
    ft = {b: transpose_apply(hps[b], ssT_s, ssT_b, 64, b, 1) for b in range(B)}

    for li, (cin, cout) in ((2, (64, 128)), (3, (128, 256))):
        ps1 = pstat.tile([1, cout], F32, tag="pstat")
        ps2 = pstat.tile([1, cout], F32, tag="pstat")
        hps = [emit_layer_cloud(b, ft[b], cin, cout, li, ps1, ps2) for b in range(B)]
        ssT_s, ssT_b = stats_to_scaleshift(ps1, ps2, cout, li)
        ft = {b: transpose_apply(hps[b], ssT_s, ssT_b, cout, b, li) for b in range(B)}

    # ---------------- embedding: streamed stats + per-channel max ----------
    # e_T chunk (oc, half) = We[:, oc]^T @ ft3[:, half]; accumulate per-channel
    # sum / sum-sq (for BN stats) and per-cloud max (for the pool) on the fly.
    acc1 = mpool.tile([128, 4], F32, tag="acc1", bufs=1)
    acc2 = mpool.tile([128, 4], F32, tag="acc2", bufs=1)
    emaxs = []
    for b in range(B):
        emax2 = mpool.tile([128, 4, 2], F32, tag="emax2", bufs=4)
        for oc in range(4):
            ocs = slice(oc * 128, (oc + 1) * 128)
            for half in range(2):
                hs = slice(half * 512, (half + 1) * 512)
                psE = pab.tile([128, 512], F32, tag="psab")
                for kc in range(2):
                    nc.tensor.matmul(psE[:], we_tiles[kc][:][:, ocs], ft[b][kc][:, hs],
                                     start=(kc == 0), stop=(kc == 1))
                red = mpool.tile([128, 3], F32, tag="red", bufs=4)
                nc.vector.tensor_reduce(red[:, 0:1], psE[:],
                                        axis=mybir.AxisListType.X, op=mybir.AluOpType.add)
                nc.vector.tensor_reduce(red[:, 2:3], psE[:],
                                        axis=mybir.AxisListType.X, op=mybir.AluOpType.max)
                et = sqpool.tile([128, 512], F32, tag="et2", bufs=1)
                nc.scalar.square(et[:], psE[:])
                nc.vector.tensor_reduce(red[:, 1:2], et[:],
                                        axis=mybir.AxisListType.X, op=mybir.AluOpType.add)
                if b == 0 and half == 0:
                    nc.vector.tensor_copy(acc1[:][:, oc:oc + 1], red[:, 0:1])
                    nc.vector.tensor_copy(acc2[:][:, oc:oc + 1], red[:, 1:2])
                else:
                    nc.vector.tensor_add(acc1[:][:, oc:oc + 1], acc1[:][:, oc:oc + 1],
                                         red[:, 0:1])
                    nc.vector.tensor_add(acc2[:][:, oc:oc + 1], acc2[:][:, oc:oc + 1],
                                         red[:, 1:2])
                nc.vector.tensor_copy(emax2[:, oc, half:half + 1], red[:, 2:3])
        emaxs.append(emax2)

    # BN scale/shift per partition-channel, [128, 4] layout
    mean = mpool.tile([128, 4], F32, tag="em_mean", bufs=1)
    ex2 = mpool.tile([128, 4], F32, tag="em_ex2", bufs=1)
    nc.vector.tensor_scalar_mul(mean[:], acc1[:], 1.0 / NPTS)
    nc.vector.tensor_scalar_mul(ex2[:], acc2[:], 1.0 / NPTS)
    msq = mpool.tile([128, 4], F32, tag="em_msq", bufs=1)
    var = mpool.tile([128, 4], F32, tag="em_var", bufs=1)
    nc.vector.tensor_mul(msq[:], mean[:], mean[:])
    nc.vector.scalar_tensor_tensor(var[:], ex2[:], EPS, msq[:],
                                   op0=mybir.AluOpType.add,
                                   op1=mybir.AluOpType.subtract)
    nc.scalar.activation(msq[:], var[:], mybir.ActivationFunctionType.Sqrt)
    nc.vector.reciprocal(var[:], msq[:])                         # 1/sqrt(var+eps)
    scale = mpool.tile([128, 4], F32, tag="em_scale", bufs=1)
    shift = mpool.tile([128, 4], F32, tag="em_shift", bufs=1)
    nc.vector.tensor_mul(scale[:], var[:], vecT_sb["ge"][:])
    nc.vector.tensor_add(msq[:], mean[:], vecT_sb["bse"][:])     # mean + bse
    nc.vector.tensor_mul(msq[:], msq[:], scale[:])
    nc.vector.tensor_sub(shift[:], vecT_sb["bee"][:], msq[:])

    # apply BN + lrelu to the per-cloud channel maxes, then emit output rows
    for b in range(B):
        em = mpool.tile([128, 4], F32, tag="em_fin", bufs=4)
        nc.vector.tensor_reduce(em[:], emaxs[b][:],
                                axis=mybir.AxisListType.X, op=mybir.AluOpType.max)
        nc.vector.tensor_mul(em[:], em[:], scale[:])
        nc.vector.tensor_add(em[:], em[:], shift[:])
        nc.vector.scalar_tensor_tensor(em[:], em[:], SLOPE, em[:],
                                       op0=mybir.AluOpType.mult,
                                       op1=mybir.AluOpType.max)
        psf = pT.tile([4, 128], F32, tag="pT")
        nc.tensor.transpose(psf[:], em[:], ident[:])
        fin = mpool.tile([4, 128], F32, tag="fin", bufs=2)
        nc.scalar.copy(fin[:], psf[:])
        dst = bass.AP(tensor=out_ap.tensor, offset=out_ap.offset + b * EMB,
                      ap=[[128, 4], [1, 128]])
        nc.sync.dma_start(dst, fin[:])


_CACHE = {}


def _build():
    if "nc" in _CACHE:
        return _CACHE["nc"]
    nc = bacc.Bacc("TRN2", target_bir_lowering=False, debug=False,
                   enable_asserts=False, num_devices=1,
                   enable_partition_id=False)
    tensors = {"blob": nc.dram_tensor("blob", [1, BLOB], F32, kind="ExternalInput"),
               "out": nc.dram_tensor("out", [B, EMB], F32, kind="ExternalOutput")}

    from contextlib import ExitStack
    with tile.TileContext(nc) as tc:
        for _rep in range(REPEAT):
            with ExitStack() as ctx:
                build_program(nc, tc, tensors, ctx)
    nc.compile()
    _CACHE["nc"] = nc
    return nc


def _in_maps(inputs):
    parts = [np.asarray(inputs["xyz"], dtype=np.float32).reshape(-1)]
    for li in (1, 2, 3):
        for nm in (f"W{li}", f"b{li}", f"g{li}", f"be{li}"):
            parts.append(np.asarray(inputs[nm], dtype=np.float32).reshape(-1))
    for nm in ("We", "bse", "ge", "bee"):
        parts.append(np.asarray(inputs[nm], dtype=np.float32).reshape(-1))
    blob = np.concatenate(parts)[None, :]
    assert blob.shape[1] == BLOB, blob.shape
    return [{"blob": np.ascontiguousarray(blob)}]


def kernel(**inputs):
    nc = _build()
    res = run_bass_kernel_spmd(nc, _in_maps(inputs), core_ids=[0])
    return np.asarray(res.results[0]["out"])
